# revision 2
# baseline (speedup 1.0000x reference)
"""Trainium2 Bass kernel for nn_Ensemble (dense MLP ensemble, E=8, B=65536).

v11 = v10 rebalanced around the real bottleneck: the PSUM->SBUF drains.
PSUM is f32 and only ACT (1.2 GHz) and DVE (0.96 GHz) can read it, both at
1 elem/cycle/lane (PSUM source forces DVE 1x mode; DMA and GPSIMD have no
PSUM port).  v10 put ~90% of the drain work on DVE (est. ~148us busy of the
160us kernel).  v11 splits EVERY 2048-col psum group between both engines,
sized so they finish together:

  - ACT drains cols [0:1128):  (172+1128)/1.2GHz = 1083 ns
  - DVE drains cols [1128:2048): (120+920)/0.96GHz = 1083 ns
  - 80 groups/member -> ~87us busy on each engine (the new floor).

PE is NOT the bottleneck (~56us busy): L1's two K=64 matmuls run
concurrently via row tile_position (0,0)/(64,0), L3's two M=64 matmuls via
col tile_position (0,0)/(0,64); only L2 (K=128) serializes.

L3 is regrouped into full 2048-col psum groups (4 col-tiled pairs) so its
drains amortize like L1/L2's.  Out-DMA ships only the 96 useful rows
(48 per batch half) -> 6 MiB instead of 8 MiB per core.
"""

import numpy as np
import ml_dtypes

BF16 = ml_dtypes.bfloat16

E = 8
B = 65536
HB = B // 2
IN = 64
AC = 16
H = 128
OUT = 48
OUTP = 64

NT = 512
SS = 512
NB = 4
T0 = (HB // SS) // NB   # 16 x-slices (ticks) per block
GW = 2048               # psum cols per group (4 banks)
SPLIT = 1128            # ACT's share of each group's drain

XW = 4096
OW = 4096
XBUFS = 3
OBUFS = 2

_CACHED = None


def _build_nc(reps=None):
    import contextlib
    import concourse.bacc as bacc
    import concourse.mybir as mybir
    import concourse.tile as tile

    f32 = mybir.dt.float32
    bf16 = mybir.dt.bfloat16
    AF = mybir.ActivationFunctionType
    ALU = mybir.AluOpType

    nc = bacc.Bacc("TRN2", target_bir_lowering=False)

    x_d = nc.dram_tensor("x", [128, HB], bf16, kind="ExternalInput")
    w1_d = nc.dram_tensor("w1p", [128, H], bf16, kind="ExternalInput")
    w2_d = nc.dram_tensor("w2", [H, H], bf16, kind="ExternalInput")
    w3_d = nc.dram_tensor("w3p", [H, OUTP], bf16, kind="ExternalInput")
    b1_d = nc.dram_tensor("b1v", [H, 1], f32, kind="ExternalInput")
    b2_d = nc.dram_tensor("b2v", [H, 1], f32, kind="ExternalInput")
    b3_d = nc.dram_tensor("b3v", [128, 1], f32, kind="ExternalInput")
    out_d = nc.dram_tensor("out", [96, HB], bf16, kind="ExternalOutput")

    BW = T0 * SS          # x cols per block (8192)
    HW_ = T0 * 2 * NT     # h cols per block (16384)

    with tile.TileContext(nc) as tc:
        with (
            tc.tile_pool(name="consts", bufs=1) as consts,
            tc.tile_pool(name="xp", bufs=XBUFS) as xp,
            tc.tile_pool(name="h1p", bufs=2) as h1pool,
            tc.tile_pool(name="h2p", bufs=2) as h2pool,
            tc.tile_pool(name="osb", bufs=OBUFS) as opool,
            tc.tile_pool(name="ps", bufs=2, space="PSUM") as psp,
        ):
            w1_sb = consts.tile([128, H], bf16)
            w2_sb = consts.tile([H, H], bf16)
            w3_sb = consts.tile([H, OUTP], bf16)
            b1_sb = consts.tile([H, 1], f32)
            b2_sb = consts.tile([H, 1], f32)
            b3_sb = consts.tile([128, 1], f32)
            nc.sync.dma_start(out=w1_sb, in_=w1_d[:])
            nc.sync.dma_start(out=w2_sb, in_=w2_d[:])
            nc.sync.dma_start(out=w3_sb, in_=w3_d[:])
            nc.sync.dma_start(out=b1_sb, in_=b1_d[:])
            nc.sync.dma_start(out=b2_sb, in_=b2_d[:])
            nc.sync.dma_start(out=b3_sb, in_=b3_d[:])

            NG = T0 // 2          # L1/L2 groups per block (8)

            loop = (tc.For_i(0, reps, 1, hint_engines=(mybir.EngineType.PE,))
                    if reps is not None else contextlib.nullcontext())
            with loop:
                h1s = {}
                h2s = {}
                for c in range(NB + 2):
                    # ---- in-DMAs for block c --------------------------
                    if c < NB:
                        x_ts = []
                        for k in range(BW // XW):
                            x_t = xp.tile([128, XW], bf16, name="x_t")
                            nc.sync.dma_start(
                                out=x_t,
                                in_=x_d[:, c * BW + k * XW:
                                        c * BW + (k + 1) * XW])
                            x_ts.append(x_t)

                    # ---- L1 phase: block c ----------------------------
                    if c < NB:
                        h1blk = h1pool.tile([128, HW_], bf16, name="h1blk")
                        for g in range(NG):
                            h1ps = psp.tile([128, GW], f32, name="h1ps", tag="gps")
                            for j in range(2):
                                i = g * 2 + j
                                x_t = x_ts[(i * SS) // XW]
                                xo = (i * SS) % XW
                                po = j * 2 * NT
                                nc.tensor.matmul(
                                    h1ps[:, po:po + NT], w1_sb[0:64, :],
                                    x_t[0:64, xo:xo + NT],
                                    start=True, stop=True,
                                    tile_position=(0, 0))
                                nc.tensor.matmul(
                                    h1ps[:, po + NT:po + 2 * NT],
                                    w1_sb[64:128, :],
                                    x_t[64:128, xo:xo + NT],
                                    start=True, stop=True,
                                    tile_position=(64, 0))
                            ho = g * GW
                            nc.scalar.activation(
                                h1blk[:, ho:ho + SPLIT], h1ps[:, 0:SPLIT],
                                AF.Relu, bias=b1_sb)
                            nc.vector.tensor_scalar(
                                h1blk[:, ho + SPLIT:ho + GW],
                                h1ps[:, SPLIT:GW], b1_sb, 0.0,
                                op0=ALU.add, op1=ALU.max)
                        h1s[c] = h1blk

                    # ---- L2 phase: block c-1 --------------------------
                    if 1 <= c <= NB:
                        h1blk = h1s.pop(c - 1)
                        h2blk = h2pool.tile([128, HW_], bf16, name="h2blk")
                        for g in range(NG):
                            h2ps = psp.tile([128, GW], f32, name="h2ps", tag="gps")
                            for j in range(2):
                                ho = (g * 2 + j) * 2 * NT
                                po = j * 2 * NT
                                nc.tensor.matmul(
                                    h2ps[:, po:po + NT], w2_sb,
                                    h1blk[:, ho:ho + NT],
                                    start=True, stop=True)
                                nc.tensor.matmul(
                                    h2ps[:, po + NT:po + 2 * NT], w2_sb,
                                    h1blk[:, ho + NT:ho + 2 * NT],
                                    start=True, stop=True)
                            ho = g * GW
                            nc.scalar.activation(
                                h2blk[:, ho:ho + SPLIT], h2ps[:, 0:SPLIT],
                                AF.Relu, bias=b2_sb)
                            nc.vector.tensor_scalar(
                                h2blk[:, ho + SPLIT:ho + GW],
                                h2ps[:, SPLIT:GW], b2_sb, 0.0,
                                op0=ALU.add, op1=ALU.max)
                        h2s[c - 1] = h2blk

                    # ---- L3 phase: block c-2, out-DMAs inline ---------
                    if c >= 2:
                        b = c - 2
                        h2blk = h2s.pop(b)
                        o_t = None
                        for g in range(BW // GW):     # 4 groups per block
                            ops = psp.tile([128, GW], f32, name="ops", tag="gps")
                            for k in range(4):
                                ho = g * 2 * GW + k * 2 * NT
                                po = k * NT
                                nc.tensor.matmul(
                                    ops[0:OUTP, po:po + NT], w3_sb,
                                    h2blk[:, ho:ho + NT],
                                    start=True, stop=True,
                                    tile_position=(0, 0))
                                nc.tensor.matmul(
                                    ops[OUTP:128, po:po + NT], w3_sb,
                                    h2blk[:, ho + NT:ho + 2 * NT],
                                    start=True, stop=True,
                                    tile_position=(0, OUTP))
                            gcol = g * GW             # out col within block
                            if gcol % OW == 0:
                                o_t = opool.tile([128, OW], bf16, name="o_t")
                            oo = gcol % OW
                            nc.scalar.add(
                                o_t[:, oo:oo + SPLIT], ops[:, 0:SPLIT],
                                b3_sb)
                            nc.vector.tensor_scalar_add(
                                o_t[:, oo + SPLIT:oo + GW],
                                ops[:, SPLIT:GW], b3_sb)
                            if (gcol + GW) % OW == 0:
                                oc = b * BW + gcol + GW - OW
                                nc.sync.dma_start(
                                    out=out_d[0:OUT, oc:oc + OW],
                                    in_=o_t[0:OUT, :])
                                nc.sync.dma_start(
                                    out=out_d[OUT:96, oc:oc + OW],
                                    in_=o_t[OUTP:OUTP + OUT, :])

    nc.compile()
    return nc


def _get_nc():
    global _CACHED
    if _CACHED is None:
        _CACHED = _build_nc()
    return _CACHED


def _prep_member(x_e, W1_e, b1_e, W2_e, b2_e, W3_e, b3_e):
    xt = np.ascontiguousarray(np.asarray(x_e).T)      # [64, B] f32
    np.clip(xt[IN - AC:IN], -1.0, 1.0, out=xt[IN - AC:IN])
    X = np.empty((128, HB), dtype=BF16)
    X[0:64] = xt[:, :HB]
    X[64:128] = xt[:, HB:]

    w1p = np.empty((128, H), dtype=BF16)
    w1p[0:64] = W1_e
    w1p[64:128] = W1_e
    w2 = W2_e.astype(BF16)
    w3p = np.zeros((H, OUTP), dtype=BF16)
    w3p[:, :OUT] = W3_e
    b1v = np.ascontiguousarray(b1_e.astype(np.float32).reshape(H, 1))
    b2v = np.ascontiguousarray(b2_e.astype(np.float32).reshape(H, 1))
    b3v = np.zeros((128, 1), dtype=np.float32)
    b3v[0:OUT, 0] = b3_e
    b3v[OUTP:OUTP + OUT, 0] = b3_e
    return {"x": X, "w1p": w1p, "w2": w2, "w3p": w3p,
            "b1v": b1v, "b2v": b2v, "b3v": b3v}


def kernel(**inputs):
    from concourse.bass_utils import run_bass_kernel_spmd

    x = np.asarray(inputs["inputs"], dtype=np.float32).reshape(E, B, IN)
    W1 = np.asarray(inputs["W1"], dtype=np.float32)
    b1 = np.asarray(inputs["b1"], dtype=np.float32)
    W2 = np.asarray(inputs["W2"], dtype=np.float32)
    b2 = np.asarray(inputs["b2"], dtype=np.float32)
    W3 = np.asarray(inputs["W3"], dtype=np.float32)
    b3 = np.asarray(inputs["b3"], dtype=np.float32)

    in_maps = [
        _prep_member(x[e], W1[e], b1[e], W2[e], b2[e], W3[e], b3[e])
        for e in range(E)
    ]

    nc = _get_nc()
    res = run_bass_kernel_spmd(nc, in_maps, core_ids=list(range(E)))

    out = np.empty((E, B, OUT), dtype=np.float32)
    for e in range(E):
        dev = res.results[e]["out"]          # [96, HB] bf16
        out[e, :HB] = dev[0:OUT, :].T
        out[e, HB:] = dev[OUT:96, :].T
    return out


# revision 5
# speedup vs baseline: 1.5296x; 1.5296x over previous
"""Trainium2 Bass kernel for nn_Ensemble (dense MLP ensemble, E=8, B=65536).

v12: concurrent ACT+DVE drains via per-engine PSUM tiles.

The bottleneck is the PSUM->SBUF drain (relu+cast): PSUM is f32, only ACT
(1.2GHz) and DVE (0.96GHz) can read it, 1 elem/cycle/lane each.  v11 split
each 2048-col psum group between both engines, but the Tile scheduler
serializes ACT and DVE accesses to the SAME psum tile (HW rule: ScE+VecE
may touch PSUM concurrently only on different banks; enforced at tile
granularity) -- the two drains ran back-to-back, not in parallel.

v12 gives each engine its own psum tile AND its own SBUF destinations:

  - ps_a [128,1024] (2 banks, x2 bufs = banks 0-3): TOP batch-half,
    drained by ACT -> h1a/h2a/o_a.
  - ps_b [128,1024] (banks 4-7): BOTTOM batch-half, drained by DVE ->
    h1b/h2b/o_b.

Per group: ACT (172+1024)/1.2 = 997ns || DVE (120+1024)/0.96 = 1192ns,
fully concurrent; 80 groups/member -> DVE-gated wall ~95us (vs 160us v10).

PE (~56us busy) stays off the critical path: L1's K=64 matmuls pack rows
(0,0)/(64,0), L3's M=64 matmuls pack cols (0,0)/(0,64) and run
concurrently in the array; only L2 (K=128) serializes.

Out-DMA ships only the 96 useful rows; o_a/o_b interleave 1024-col chunks
in HBM via 3D (rearranged) DMA APs.
"""

import numpy as np
import ml_dtypes

BF16 = ml_dtypes.bfloat16

E = 8
B = 65536
HB = B // 2
IN = 64
AC = 16
H = 128
OUT = 48
OUTP = 64

NT = 512
NB = 4
GW = 1024               # psum cols per engine-group (2 banks)

XW = 4096
XBUFS = 3
OBUFS = 2

_CACHED = None


def _build_nc(reps=None):
    import contextlib
    import concourse.bacc as bacc
    import concourse.mybir as mybir
    import concourse.tile as tile

    f32 = mybir.dt.float32
    bf16 = mybir.dt.bfloat16
    AF = mybir.ActivationFunctionType
    ALU = mybir.AluOpType

    nc = bacc.Bacc("TRN2", target_bir_lowering=False)

    x_d = nc.dram_tensor("x", [128, HB], bf16, kind="ExternalInput")
    w1_d = nc.dram_tensor("w1p", [128, H], bf16, kind="ExternalInput")
    w2_d = nc.dram_tensor("w2", [H, H], bf16, kind="ExternalInput")
    w3_d = nc.dram_tensor("w3p", [H, OUTP], bf16, kind="ExternalInput")
    b1_d = nc.dram_tensor("b1v", [H, 1], f32, kind="ExternalInput")
    b2_d = nc.dram_tensor("b2v", [H, 1], f32, kind="ExternalInput")
    b3_d = nc.dram_tensor("b3v", [128, 1], f32, kind="ExternalInput")
    out_d = nc.dram_tensor("out", [96, HB], bf16, kind="ExternalOutput")

    BW = 8192             # x cols per block (HB / NB)
    HW_ = 8192            # h cols per half-block tile

    with tile.TileContext(nc) as tc:
        with (
            tc.tile_pool(name="consts", bufs=1) as consts,
            tc.tile_pool(name="xp", bufs=XBUFS) as xp,
            tc.tile_pool(name="h1a", bufs=2) as h1ap,
            tc.tile_pool(name="h1b", bufs=2) as h1bp,
            tc.tile_pool(name="h2a", bufs=2) as h2ap,
            tc.tile_pool(name="h2b", bufs=2) as h2bp,
            tc.tile_pool(name="oa", bufs=OBUFS) as oap,
            tc.tile_pool(name="ob", bufs=OBUFS) as obp,
            tc.tile_pool(name="psa", bufs=2, space="PSUM") as psap,
            tc.tile_pool(name="psb", bufs=2, space="PSUM") as psbp,
        ):
            w1_sb = consts.tile([128, H], bf16)
            w2_sb = consts.tile([H, H], bf16)
            w3_sb = consts.tile([H, OUTP], bf16)
            b1_sb = consts.tile([H, 1], f32)
            b2_sb = consts.tile([H, 1], f32)
            b3_sb = consts.tile([128, 1], f32)
            nc.sync.dma_start(out=w1_sb, in_=w1_d[:])
            nc.sync.dma_start(out=w2_sb, in_=w2_d[:])
            nc.sync.dma_start(out=w3_sb, in_=w3_d[:])
            nc.sync.dma_start(out=b1_sb, in_=b1_d[:])
            nc.sync.dma_start(out=b2_sb, in_=b2_d[:])
            nc.sync.dma_start(out=b3_sb, in_=b3_d[:])

            loop = (tc.For_i(0, reps, 1, hint_engines=(mybir.EngineType.PE,))
                    if reps is not None else contextlib.nullcontext())
            with loop:
                h1s = {}
                h2s = {}
                for c in range(NB + 2):
                    # ---- in-DMAs for block c --------------------------
                    if c < NB:
                        x_ts = []
                        for k in range(BW // XW):
                            x_t = xp.tile([128, XW], bf16, name="x_t")
                            nc.sync.dma_start(
                                out=x_t,
                                in_=x_d[:, c * BW + k * XW:
                                        c * BW + (k + 1) * XW])
                            x_ts.append(x_t)

                    # ---- L1 phase: block c ----------------------------
                    if c < NB:
                        h1a = h1ap.tile([128, HW_], bf16, name="h1a")
                        h1b = h1bp.tile([128, HW_], bf16, name="h1b")
                        for g in range(8):
                            ps_a = psap.tile([128, GW], f32, name="psa", tag="psa")
                            ps_b = psbp.tile([128, GW], f32, name="psb", tag="psb")
                            x_t = x_ts[(g * GW) // XW]
                            xo = (g * GW) % XW
                            for j in range(2):
                                nc.tensor.matmul(
                                    ps_a[:, j * NT:(j + 1) * NT],
                                    w1_sb[0:64, :],
                                    x_t[0:64, xo + j * NT:xo + (j + 1) * NT],
                                    start=True, stop=True,
                                    tile_position=(0, 0))
                                nc.tensor.matmul(
                                    ps_b[:, j * NT:(j + 1) * NT],
                                    w1_sb[64:128, :],
                                    x_t[64:128, xo + j * NT:xo + (j + 1) * NT],
                                    start=True, stop=True,
                                    tile_position=(64, 0))
                            ho = g * GW
                            nc.scalar.activation(
                                h1a[:, ho:ho + GW], ps_a,
                                AF.Relu, bias=b1_sb)
                            nc.vector.tensor_scalar(
                                h1b[:, ho:ho + GW], ps_b, b1_sb, 0.0,
                                op0=ALU.add, op1=ALU.max)
                        h1s[c] = (h1a, h1b)

                    # ---- L2 phase: block c-1 --------------------------
                    if 1 <= c <= NB:
                        h1a, h1b = h1s.pop(c - 1)
                        h2a = h2ap.tile([128, HW_], bf16, name="h2a")
                        h2b = h2bp.tile([128, HW_], bf16, name="h2b")
                        for g in range(8):
                            ps_a = psap.tile([128, GW], f32, name="psa", tag="psa")
                            ps_b = psbp.tile([128, GW], f32, name="psb", tag="psb")
                            ho = g * GW
                            for j in range(2):
                                nc.tensor.matmul(
                                    ps_a[:, j * NT:(j + 1) * NT], w2_sb,
                                    h1a[:, ho + j * NT:ho + (j + 1) * NT],
                                    start=True, stop=True)
                                nc.tensor.matmul(
                                    ps_b[:, j * NT:(j + 1) * NT], w2_sb,
                                    h1b[:, ho + j * NT:ho + (j + 1) * NT],
                                    start=True, stop=True)
                            nc.scalar.activation(
                                h2a[:, ho:ho + GW], ps_a,
                                AF.Relu, bias=b2_sb)
                            nc.vector.tensor_scalar(
                                h2b[:, ho:ho + GW], ps_b, b2_sb, 0.0,
                                op0=ALU.add, op1=ALU.max)
                        h2s[c - 1] = (h2a, h2b)

                    # ---- L3 phase: block c-2, out-DMAs at block end ---
                    if c >= 2:
                        b = c - 2
                        h2a, h2b = h2s.pop(b)
                        o_a = oap.tile([128, 4096], bf16, name="o_a")
                        o_b = obp.tile([128, 4096], bf16, name="o_b")
                        for g in range(4):
                            ps_a = psap.tile([128, GW], f32, name="psa", tag="psa")
                            ps_b = psbp.tile([128, GW], f32, name="psb", tag="psb")
                            for k, ps in ((0, ps_a), (1, ps_b)):
                                co = (2 * g + k) * GW
                                for j in range(2):
                                    nc.tensor.matmul(
                                        ps[0:OUTP, j * NT:(j + 1) * NT],
                                        w3_sb,
                                        h2a[:, co + j * NT:co + (j + 1) * NT],
                                        start=True, stop=True,
                                        tile_position=(0, 0))
                                    nc.tensor.matmul(
                                        ps[OUTP:128, j * NT:(j + 1) * NT],
                                        w3_sb,
                                        h2b[:, co + j * NT:co + (j + 1) * NT],
                                        start=True, stop=True,
                                        tile_position=(0, OUTP))
                            oo = g * GW
                            nc.scalar.add(o_a[:, oo:oo + GW], ps_a, b3_sb)
                            nc.vector.tensor_scalar_add(
                                o_b[:, oo:oo + GW], ps_b, b3_sb)
                        base = b * BW
                        dst = out_d[:, base:base + BW].rearrange(
                            "p (g t) -> p g t", t=2 * GW)
                        for rows, orows in ((slice(0, OUT), slice(0, OUT)),
                                            (slice(OUT, 96),
                                             slice(OUTP, OUTP + OUT))):
                            nc.sync.dma_start(
                                out=dst[rows, :, 0:GW],
                                in_=o_a[orows, :].rearrange(
                                    "p (g t) -> p g t", t=GW))
                            nc.sync.dma_start(
                                out=dst[rows, :, GW:2 * GW],
                                in_=o_b[orows, :].rearrange(
                                    "p (g t) -> p g t", t=GW))

    nc.compile()
    return nc


def _get_nc():
    global _CACHED
    if _CACHED is None:
        _CACHED = _build_nc()
    return _CACHED


def _prep_member(x_e, W1_e, b1_e, W2_e, b2_e, W3_e, b3_e):
    xt = np.ascontiguousarray(np.asarray(x_e).T)      # [64, B] f32
    np.clip(xt[IN - AC:IN], -1.0, 1.0, out=xt[IN - AC:IN])
    X = np.empty((128, HB), dtype=BF16)
    X[0:64] = xt[:, :HB]
    X[64:128] = xt[:, HB:]

    w1p = np.empty((128, H), dtype=BF16)
    w1p[0:64] = W1_e
    w1p[64:128] = W1_e
    w2 = W2_e.astype(BF16)
    w3p = np.zeros((H, OUTP), dtype=BF16)
    w3p[:, :OUT] = W3_e
    b1v = np.ascontiguousarray(b1_e.astype(np.float32).reshape(H, 1))
    b2v = np.ascontiguousarray(b2_e.astype(np.float32).reshape(H, 1))
    b3v = np.zeros((128, 1), dtype=np.float32)
    b3v[0:OUT, 0] = b3_e
    b3v[OUTP:OUTP + OUT, 0] = b3_e
    return {"x": X, "w1p": w1p, "w2": w2, "w3p": w3p,
            "b1v": b1v, "b2v": b2v, "b3v": b3v}


def kernel(**inputs):
    from concourse.bass_utils import run_bass_kernel_spmd

    x = np.asarray(inputs["inputs"], dtype=np.float32).reshape(E, B, IN)
    W1 = np.asarray(inputs["W1"], dtype=np.float32)
    b1 = np.asarray(inputs["b1"], dtype=np.float32)
    W2 = np.asarray(inputs["W2"], dtype=np.float32)
    b2 = np.asarray(inputs["b2"], dtype=np.float32)
    W3 = np.asarray(inputs["W3"], dtype=np.float32)
    b3 = np.asarray(inputs["b3"], dtype=np.float32)

    in_maps = [
        _prep_member(x[e], W1[e], b1[e], W2[e], b2[e], W3[e], b3[e])
        for e in range(E)
    ]

    nc = _get_nc()
    res = run_bass_kernel_spmd(nc, in_maps, core_ids=list(range(E)))

    out = np.empty((E, B, OUT), dtype=np.float32)
    for e in range(E):
        dev = res.results[e]["out"]          # [96, HB] bf16
        out[e, :HB] = dev[0:OUT, :].T
        out[e, HB:] = dev[OUT:96, :].T
    return out


# revision 7
# speedup vs baseline: 1.8044x; 1.1796x over previous
"""Trainium2 Bass kernel for nn_Ensemble (dense MLP ensemble, E=8, B=65536).

v13 = v12 (concurrent ACT+DVE drains via per-engine PSUM tiles) with:

  (i) tick-interleaved phases: instead of [8x L1][8x L2][4x L3] per block
      cycle, emit per tick t: L1 g_t | L2 g_t | (odd t) L3 g_{t//2}.
      The drain engines see a uniform supply -> no phase-edge bubbles,
      and the loop tail (L3-only) shrinks.
 (ii) ACT steals 7 of DVE's 80 b-side drains (ACT 997ns/call vs DVE
      1192ns/call): ACT ~87us ~= DVE ~87us busy, the engine floor.

Background (v12): PSUM is f32; only ACT+DVE can read it (1 elem/cyc/lane).
The Tile scheduler serializes ACT and DVE touching the SAME psum tile, so
each engine gets its own: ps_a (banks 0-3, TOP batch-half -> ACT ->
h1a/h2a/o_a) and ps_b (banks 4-7, BOTTOM half -> DVE -> h1b/h2b/o_b).
PE packs L1's K=64 matmuls in rows (0,0)/(64,0) and L3's M=64 in cols
(0,0)/(0,64) (concurrent in-array), so PE (~56us) is off the critical
path.  Out-DMA ships only the 96 useful rows (6 MiB/core).
"""

import numpy as np
import ml_dtypes

BF16 = ml_dtypes.bfloat16

E = 8
B = 65536
HB = B // 2
IN = 64
AC = 16
H = 128
OUT = 48
OUTP = 64

NT = 512
NB = 4
GW = 1024               # psum cols per engine-group (2 banks)

XW = 4096
XBUFS = 3
OBUFS = 2

# (phase, block, tick) triples whose b-side drain ACT steals from DVE:
# 7 per pass, spread across blocks; phase 1=L1, 2=L2.
_STEAL = {(1, 0, 2), (1, 1, 6), (1, 2, 4), (1, 3, 0),
          (2, 0, 3), (2, 1, 5), (2, 2, 1)}

_CACHED = None


def _build_nc(reps=None):
    import contextlib
    import concourse.bacc as bacc
    import concourse.mybir as mybir
    import concourse.tile as tile

    f32 = mybir.dt.float32
    bf16 = mybir.dt.bfloat16
    AF = mybir.ActivationFunctionType
    ALU = mybir.AluOpType

    nc = bacc.Bacc("TRN2", target_bir_lowering=False)

    x_d = nc.dram_tensor("x", [128, HB], bf16, kind="ExternalInput")
    w1_d = nc.dram_tensor("w1p", [128, H], bf16, kind="ExternalInput")
    w2_d = nc.dram_tensor("w2", [H, H], bf16, kind="ExternalInput")
    w3_d = nc.dram_tensor("w3p", [H, OUTP], bf16, kind="ExternalInput")
    b1_d = nc.dram_tensor("b1v", [H, 1], f32, kind="ExternalInput")
    b2_d = nc.dram_tensor("b2v", [H, 1], f32, kind="ExternalInput")
    b3_d = nc.dram_tensor("b3v", [128, 1], f32, kind="ExternalInput")
    out_d = nc.dram_tensor("out", [96, HB], bf16, kind="ExternalOutput")

    BW = 8192             # x cols per block (HB / NB)
    HW_ = 8192            # h cols per half-block tile

    with tile.TileContext(nc) as tc:
        with (
            tc.tile_pool(name="consts", bufs=1) as consts,
            tc.tile_pool(name="xp", bufs=XBUFS) as xp,
            tc.tile_pool(name="h1a", bufs=2) as h1ap,
            tc.tile_pool(name="h1b", bufs=2) as h1bp,
            tc.tile_pool(name="h2a", bufs=2) as h2ap,
            tc.tile_pool(name="h2b", bufs=2) as h2bp,
            tc.tile_pool(name="oa", bufs=OBUFS) as oap,
            tc.tile_pool(name="ob", bufs=OBUFS) as obp,
            tc.tile_pool(name="psa", bufs=2, space="PSUM") as psap,
            tc.tile_pool(name="psb", bufs=2, space="PSUM") as psbp,
        ):
            w1_sb = consts.tile([128, H], bf16)
            w2_sb = consts.tile([H, H], bf16)
            w3_sb = consts.tile([H, OUTP], bf16)
            b1_sb = consts.tile([H, 1], f32)
            b2_sb = consts.tile([H, 1], f32)
            b3_sb = consts.tile([128, 1], f32)
            nc.sync.dma_start(out=w1_sb, in_=w1_d[:])
            nc.sync.dma_start(out=w2_sb, in_=w2_d[:])
            nc.sync.dma_start(out=w3_sb, in_=w3_d[:])
            nc.sync.dma_start(out=b1_sb, in_=b1_d[:])
            nc.sync.dma_start(out=b2_sb, in_=b2_d[:])
            nc.sync.dma_start(out=b3_sb, in_=b3_d[:])

            def l1_group(x_ts, h1a, h1b, g, blk):
                ps_a = psap.tile([128, GW], f32, name="psa", tag="psa")
                ps_b = psbp.tile([128, GW], f32, name="psb", tag="psb")
                x_t = x_ts[(g * GW) // XW]
                xo = (g * GW) % XW
                for j in range(2):
                    nc.tensor.matmul(
                        ps_a[:, j * NT:(j + 1) * NT], w1_sb[0:64, :],
                        x_t[0:64, xo + j * NT:xo + (j + 1) * NT],
                        start=True, stop=True, tile_position=(0, 0))
                    nc.tensor.matmul(
                        ps_b[:, j * NT:(j + 1) * NT], w1_sb[64:128, :],
                        x_t[64:128, xo + j * NT:xo + (j + 1) * NT],
                        start=True, stop=True, tile_position=(64, 0))
                ho = g * GW
                nc.scalar.activation(h1a[:, ho:ho + GW], ps_a,
                                     AF.Relu, bias=b1_sb)
                if (1, blk, g) in _STEAL:
                    nc.scalar.activation(h1b[:, ho:ho + GW], ps_b,
                                         AF.Relu, bias=b1_sb)
                else:
                    nc.vector.tensor_scalar(h1b[:, ho:ho + GW], ps_b,
                                            b1_sb, 0.0,
                                            op0=ALU.add, op1=ALU.max)

            def l2_group(h1a, h1b, h2a, h2b, g, blk):
                ps_a = psap.tile([128, GW], f32, name="psa", tag="psa")
                ps_b = psbp.tile([128, GW], f32, name="psb", tag="psb")
                ho = g * GW
                for j in range(2):
                    nc.tensor.matmul(
                        ps_a[:, j * NT:(j + 1) * NT], w2_sb,
                        h1a[:, ho + j * NT:ho + (j + 1) * NT],
                        start=True, stop=True)
                    nc.tensor.matmul(
                        ps_b[:, j * NT:(j + 1) * NT], w2_sb,
                        h1b[:, ho + j * NT:ho + (j + 1) * NT],
                        start=True, stop=True)
                nc.scalar.activation(h2a[:, ho:ho + GW], ps_a,
                                     AF.Relu, bias=b2_sb)
                if (2, blk, g) in _STEAL:
                    nc.scalar.activation(h2b[:, ho:ho + GW], ps_b,
                                         AF.Relu, bias=b2_sb)
                else:
                    nc.vector.tensor_scalar(h2b[:, ho:ho + GW], ps_b,
                                            b2_sb, 0.0,
                                            op0=ALU.add, op1=ALU.max)

            def l3_group(h2a, h2b, o_a, o_b, g):
                ps_a = psap.tile([128, GW], f32, name="psa", tag="psa")
                ps_b = psbp.tile([128, GW], f32, name="psb", tag="psb")
                for k, ps in ((0, ps_a), (1, ps_b)):
                    co = (2 * g + k) * GW
                    for j in range(2):
                        nc.tensor.matmul(
                            ps[0:OUTP, j * NT:(j + 1) * NT], w3_sb,
                            h2a[:, co + j * NT:co + (j + 1) * NT],
                            start=True, stop=True, tile_position=(0, 0))
                        nc.tensor.matmul(
                            ps[OUTP:128, j * NT:(j + 1) * NT], w3_sb,
                            h2b[:, co + j * NT:co + (j + 1) * NT],
                            start=True, stop=True, tile_position=(0, OUTP))
                oo = g * GW
                nc.scalar.add(o_a[:, oo:oo + GW], ps_a, b3_sb)
                nc.vector.tensor_scalar_add(o_b[:, oo:oo + GW], ps_b, b3_sb)

            def out_dmas(o_a, o_b, b):
                base = b * BW
                dst = out_d[:, base:base + BW].rearrange(
                    "p (g t) -> p g t", t=2 * GW)
                for rows, orows in ((slice(0, OUT), slice(0, OUT)),
                                    (slice(OUT, 96),
                                     slice(OUTP, OUTP + OUT))):
                    nc.sync.dma_start(
                        out=dst[rows, :, 0:GW],
                        in_=o_a[orows, :].rearrange("p (g t) -> p g t", t=GW))
                    nc.sync.dma_start(
                        out=dst[rows, :, GW:2 * GW],
                        in_=o_b[orows, :].rearrange("p (g t) -> p g t", t=GW))

            loop = (tc.For_i(0, reps, 1, hint_engines=(mybir.EngineType.PE,))
                    if reps is not None else contextlib.nullcontext())
            with loop:
                h1s = {}
                h2s = {}
                for c in range(NB + 2):
                    if c < NB:
                        x_ts = []
                        for k in range(BW // XW):
                            x_t = xp.tile([128, XW], bf16, name="x_t")
                            nc.sync.dma_start(
                                out=x_t,
                                in_=x_d[:, c * BW + k * XW:
                                        c * BW + (k + 1) * XW])
                            x_ts.append(x_t)
                        h1a = h1ap.tile([128, HW_], bf16, name="h1a")
                        h1b = h1bp.tile([128, HW_], bf16, name="h1b")
                        h1s[c] = (h1a, h1b)
                    if 1 <= c <= NB:
                        p1a, p1b = h1s[c - 1]
                        h2a = h2ap.tile([128, HW_], bf16, name="h2a")
                        h2b = h2bp.tile([128, HW_], bf16, name="h2b")
                        h2s[c - 1] = (h2a, h2b)
                    if c >= 2:
                        p2a, p2b = h2s[c - 2]
                        o_a = oap.tile([128, 4096], bf16, name="o_a")
                        o_b = obp.tile([128, 4096], bf16, name="o_b")

                    # interleave the three phases tick by tick
                    for t in range(8):
                        if c < NB:
                            l1_group(x_ts, h1a, h1b, t, c)
                        if 1 <= c <= NB:
                            l2_group(p1a, p1b, h2a, h2b, t, c - 1)
                        if c >= 2 and t % 2 == 1:
                            l3_group(p2a, p2b, o_a, o_b, t // 2)

                    if c >= 2:
                        out_dmas(o_a, o_b, c - 2)
                        h2s.pop(c - 2)
                    if 1 <= c <= NB:
                        h1s.pop(c - 1)

    nc.compile()
    return nc


def _get_nc():
    global _CACHED
    if _CACHED is None:
        _CACHED = _build_nc()
    return _CACHED


def _prep_member(x_e, W1_e, b1_e, W2_e, b2_e, W3_e, b3_e):
    xt = np.ascontiguousarray(np.asarray(x_e).T)      # [64, B] f32
    np.clip(xt[IN - AC:IN], -1.0, 1.0, out=xt[IN - AC:IN])
    X = np.empty((128, HB), dtype=BF16)
    X[0:64] = xt[:, :HB]
    X[64:128] = xt[:, HB:]

    w1p = np.empty((128, H), dtype=BF16)
    w1p[0:64] = W1_e
    w1p[64:128] = W1_e
    w2 = W2_e.astype(BF16)
    w3p = np.zeros((H, OUTP), dtype=BF16)
    w3p[:, :OUT] = W3_e
    b1v = np.ascontiguousarray(b1_e.astype(np.float32).reshape(H, 1))
    b2v = np.ascontiguousarray(b2_e.astype(np.float32).reshape(H, 1))
    b3v = np.zeros((128, 1), dtype=np.float32)
    b3v[0:OUT, 0] = b3_e
    b3v[OUTP:OUTP + OUT, 0] = b3_e
    return {"x": X, "w1p": w1p, "w2": w2, "w3p": w3p,
            "b1v": b1v, "b2v": b2v, "b3v": b3v}


def kernel(**inputs):
    from concourse.bass_utils import run_bass_kernel_spmd

    x = np.asarray(inputs["inputs"], dtype=np.float32).reshape(E, B, IN)
    W1 = np.asarray(inputs["W1"], dtype=np.float32)
    b1 = np.asarray(inputs["b1"], dtype=np.float32)
    W2 = np.asarray(inputs["W2"], dtype=np.float32)
    b2 = np.asarray(inputs["b2"], dtype=np.float32)
    W3 = np.asarray(inputs["W3"], dtype=np.float32)
    b3 = np.asarray(inputs["b3"], dtype=np.float32)

    in_maps = [
        _prep_member(x[e], W1[e], b1[e], W2[e], b2[e], W3[e], b3[e])
        for e in range(E)
    ]

    nc = _get_nc()
    res = run_bass_kernel_spmd(nc, in_maps, core_ids=list(range(E)))

    out = np.empty((E, B, OUT), dtype=np.float32)
    for e in range(E):
        dev = res.results[e]["out"]          # [96, HB] bf16
        out[e, :HB] = dev[0:OUT, :].T
        out[e, HB:] = dev[OUT:96, :].T
    return out
